# revision 19
# baseline (speedup 1.0000x reference)
"""DeepseekV2-Lite MLA-vanilla attention block on 8 Trainium2 NeuronCores.

Sharding: tensor-parallel over the 16 heads (4 groups of 4 heads) x
data-parallel over batch (2) -> 8 cores. The kv_a (compressed latent) path is
replicated within a batch. Each core computes a partial output
(its 4 heads' contribution through Wo); the host sums the 4 partials per batch.

All on-device layouts are feature-major ("transposed") so every matmul
contracts over the SBUF partition dimension:
  - hsT [HID, S], wqT [HID, 768], ... prepared host-side.
  - scores are computed transposed, sT[j, i] = k . q, so softmax row sums are
    ones-vector matmuls and the causal mask is 4 static diagonal tiles.
  - softmax skips max-subtraction (scores are O(1) for these inputs; exp is
    computed in fp32 which is safe up to ~80).
Matmuls run as float32r (~1e-4 relative error at K=2048).
"""

import os
import sys
from contextlib import contextmanager

sys.path.insert(0, "/opt/trn_rl_repo")

_PHASES = int(os.environ.get("KPHASES", "4"))  # debug: 1=A, 2=+B1, 3=+B2, 4=full


@contextmanager
def _maybe(cond):
    yield cond

import numpy as np
import ml_dtypes

import concourse.bass as bass  # noqa: F401  (AP types)
import concourse.mybir as mybir
import concourse.tile as tile
from concourse import bacc
from concourse.bass_utils import run_bass_kernel_spmd

B, S, HID = 2, 2048, 2048
NH, D_NOPE, D_ROPE, D_Q, D_V, LORA = 16, 128, 64, 192, 128, 512
SCALE = D_Q ** -0.5
EPS = 1e-6
G = 4          # head groups (tensor parallel)
HPG = NH // G  # heads per group
N_CORES = 8
NT = S // 512  # 512-token tiles
TT = S // 128  # 128-token tiles

TRACE = False  # set by test.py to capture an NTFF profile

f32 = mybir.dt.float32
f32r = mybir.dt.float32r
bf16 = mybir.dt.bfloat16

_compiled = None


def _build():
    FT = mybir.ActivationFunctionType
    OP = mybir.AluOpType

    nc = bacc.Bacc("TRN2", target_bir_lowering=False, debug=False,
                   num_devices=N_CORES)

    hsT = nc.dram_tensor("hsT", [HID, S], f32r, kind="ExternalInput").ap()
    wqT = nc.dram_tensor("wqT", [HID, HPG * D_Q], f32r, kind="ExternalInput").ap()
    wkvaT = nc.dram_tensor("wkvaT", [HID, LORA + D_ROPE], f32r, kind="ExternalInput").ap()
    wkvbkT = nc.dram_tensor("wkvbkT", [LORA, HPG * D_NOPE], f32r, kind="ExternalInput").ap()
    wkvbvT = nc.dram_tensor("wkvbvT", [LORA, HPG * D_V], f32r, kind="ExternalInput").ap()
    woT = nc.dram_tensor("woT", [HPG * D_V, HID], f32r, kind="ExternalInput").ap()
    normw = nc.dram_tensor("normw", [128, 4], f32, kind="ExternalInput").ap()
    cs = nc.dram_tensor("cs", [128, 2], f32, kind="ExternalInput").ap()
    masks = nc.dram_tensor("masks", [128, 4, 512], bf16, kind="ExternalInput").ap()
    onec = nc.dram_tensor("onec", [128, 1], f32r, kind="ExternalInput").ap()
    oner = nc.dram_tensor("oner", [1, 128], f32r, kind="ExternalInput").ap()
    outp = nc.dram_tensor("outp", [S, HID], f32, kind="ExternalOutput").ap()

    hsT_r = hsT.rearrange("(ko p) t -> p ko t", p=128)        # [128, 16, S]
    wqT_r = wqT.rearrange("(ko p) f -> p ko f", p=128)        # [128, 16, 768]
    wkvaT_r = wkvaT.rearrange("(ko p) f -> p ko f", p=128)    # [128, 16, 576]
    wkvbkT_r = wkvbkT.rearrange("(c p) f -> p c f", p=128)    # [128, 4, 512]
    wkvbvT_r = wkvbvT.rearrange("(c p) f -> p c f", p=128)    # [128, 4, 512]
    woT_r = woT.rearrange("(c p) o -> p c o", p=128)          # [128, 4, HID]

    with tile.TileContext(nc) as tc, nc.allow_low_precision(
        reason="float32r rounding of matmul operands is the design"
    ):
        with (
            tc.tile_pool(name="dram", bufs=1, space="DRAM") as dram,
            tc.tile_pool(name="const", bufs=1) as const,
            tc.tile_pool(name="keep", bufs=1) as keep,
        ):
            knope_dram = dram.tile([128, HPG, S], f32r)   # [d_nope, head, t]
            ao_dram = dram.tile([128, HPG, S], f32r)      # [d_v, head, t]

            c_onec = const.tile([128, 1], f32r)
            nc.sync.dma_start(c_onec[:], onec)
            c_oner = const.tile([1, 128], f32r)
            nc.sync.dma_start(c_oner[:], oner)
            c_cs = const.tile([128, 2], f32)
            nc.sync.dma_start(c_cs[:], cs)
            c_normw = const.tile([128, 4], f32)
            nc.sync.dma_start(c_normw[:], normw)
            c_masks = const.tile([128, 4, 512], bf16)
            nc.sync.dma_start(c_masks[:], masks)
            c_eps = const.tile([1, 1], f32)
            nc.gpsimd.memset(c_eps[:], EPS)

            # k_pe stored twice (partitions 0:64 and 64:128) so the scores
            # matmul lhsT base_partition can match either q_pe half.
            kpeT = keep.tile([128, S], f32r, tag="kpeT")
            qT = keep.tile([128, 6, S], f32r, tag="qT")
            v_dram = dram.tile([128, TT, HPG * D_V], f32r)  # [tok, tt, dv]

            # ---------- Phase A: ckv projection, RMSNorm, kv_b ----------
            M_CKV = ((0, 128), (128, 128), (256, 128), (384, 128), (512, 64))
            if True:
                with tc.tile_pool(name="ckv", bufs=1) as ckvp:
                    # f32r so the RMSNorm can run in place and kv_b can
                    # consume it directly.
                    ckv = ckvp.tile([128, 5, S], f32r)
                    with (
                        tc.tile_pool(name="wkva", bufs=1) as wk,
                        tc.tile_pool(name="hsA", bufs=2) as hsp,
                        tc.tile_pool(name="psA", bufs=6, space="PSUM") as psA,
                    ):
                        wkva_sb = wk.tile([128, 16, 576], f32r)
                        for k in range(16):
                            nc.sync.dma_start(wkva_sb[:, k], wkvaT_r[:, k])
                        for nt in range(NT):
                            nts = slice(nt * 512, (nt + 1) * 512)
                            pms = [psA.tile([128, 512], f32, tag="pa", name=f"pa{m}")
                                   for m in range(5)]
                            for ko in range(4):
                                hq = hsp.tile([128, 4, 512], f32r, tag="hs")
                                for kk in range(4):
                                    nc.sync.dma_start(hq[:, kk], hsT_r[:, ko * 4 + kk, nts])
                                for kk in range(4):
                                    k = ko * 4 + kk
                                    for m, (mo, mw) in enumerate(M_CKV):
                                        nc.tensor.matmul(
                                            pms[m][:mw], wkva_sb[:, k, mo:mo + mw],
                                            hq[:, kk], start=(k == 0), stop=(k == 15))
                            for m, (mo, mw) in enumerate(M_CKV):
                                nc.scalar.copy(ckv[:mw, m, nts], pms[m][:mw])

                    # RoPE on k_pe (rows 0:64 of chunk 4); cos/sin broadcast
                    # over tokens, so they are per-partition scalars here.
                    with tc.tile_pool(name="ropek", bufs=1) as rkp:
                        rk = rkp.tile([64, S], f32)
                        nc.scalar.copy(rk[0:32], ckv[32:64, 4, :])
                        nc.scalar.copy(rk[32:64], ckv[0:32, 4, :])
                        nc.vector.tensor_scalar_mul(kpeT[0:64], ckv[0:64, 4, :], c_cs[0:64, 0:1])
                        nc.vector.tensor_scalar_mul(rk[:], rk[:], c_cs[0:64, 1:2])
                        nc.vector.tensor_add(kpeT[0:64], kpeT[0:64], rk[:])
                        nc.scalar.copy(kpeT[64:128], kpeT[0:64])

                    # RMSNorm over the 512 latent channels (4 partition
                    # chunks), in place on ckv chunks 0..3.
                    with (
                        tc.tile_pool(name="ntmp", bufs=2) as ntp,
                        tc.tile_pool(name="psN", bufs=2, space="PSUM") as psN,
                        tc.tile_pool(name="psNb", bufs=2, space="PSUM") as psNb,
                    ):
                        for nt in range(NT):
                            nts = slice(nt * 512, (nt + 1) * 512)
                            ssq = psN.tile([1, 512], f32, tag="ssq")
                            for c in range(4):
                                sq = ntp.tile([128, 512], f32r, tag="sq")
                                nc.scalar.activation(sq[:], ckv[:, c, nts], FT.Square)
                                nc.tensor.matmul(ssq[:], c_onec[:], sq[:],
                                                 start=(c == 0), stop=(c == 3))
                            rms = ntp.tile([1, 512], f32, tag="rms")
                            nc.scalar.activation(rms[:], ssq[:], FT.Sqrt,
                                                 scale=1.0 / LORA, bias=c_eps[:])
                            rinv = ntp.tile([1, 512], f32r, tag="rinv")
                            nc.vector.reciprocal(rinv[:], rms[:])
                            bc = psNb.tile([128, 512], f32, tag="bc")
                            nc.tensor.matmul(bc[:], c_oner[:], rinv[:], start=True, stop=True)
                            for c in range(4):
                                nc.vector.tensor_tensor(ckv[:, c, nts],
                                                        ckv[:, c, nts], bc[:], OP.mult)
                                nc.vector.tensor_scalar_mul(ckv[:, c, nts],
                                                            ckv[:, c, nts],
                                                            c_normw[:, c:c + 1])

                    # kv_b projections (ckv now holds the normalized latent)
                    with (
                        tc.tile_pool(name="wkvb", bufs=1) as wbp,
                        tc.tile_pool(name="kvst", bufs=3) as kst,
                        tc.tile_pool(name="psB", bufs=4, space="PSUM") as psB,
                    ):
                        wbk = wbp.tile([128, 4, 512], f32r, tag="wbk")
                        wbv = wbp.tile([128, 4, 512], f32r, tag="wbv")
                        for c in range(4):
                            nc.sync.dma_start(wbk[:, c], wkvbkT_r[:, c])
                            nc.sync.dma_start(wbv[:, c], wkvbvT_r[:, c])
                        # k_nope, feature-major: [d_nope, head, t]
                        for m in range(HPG):
                            for nt in range(NT):
                                nts = slice(nt * 512, (nt + 1) * 512)
                                pm = psB.tile([128, 512], f32, tag="pb")
                                for c in range(4):
                                    nc.tensor.matmul(pm[:], wbk[:, c, m * 128:(m + 1) * 128],
                                                     ckv[:, c, nts],
                                                     start=(c == 0), stop=(c == 3))
                                ks = kst.tile([128, 512], f32r, tag="kn")
                                nc.scalar.copy(ks[:], pm[:])
                                nc.sync.dma_start(knope_dram[:, m, nts], ks[:])
                        # v, token-major: [tok, tt, dv]
                        for tt in range(TT):
                            tts = slice(tt * 128, (tt + 1) * 128)
                            pv = psB.tile([128, 512], f32, tag="pb")
                            for c in range(4):
                                nc.tensor.matmul(pv[:], ckv[:, c, tts], wbv[:, c, :],
                                                 start=(c == 0), stop=(c == 3))
                            vs = kst.tile([128, 512], f32r, tag="vs")
                            nc.scalar.copy(vs[:], pv[:])
                            nc.sync.dma_start(v_dram[:, tt, :], vs[:])

            # ---------- Phase B1: q projection (+ scale + RoPE) ----------
            with (  # noqa: SIM117
                _maybe(_PHASES >= 2) as _go2,
            ):
             if _go2:
              with (
                tc.tile_pool(name="wq", bufs=1) as wqp,
                tc.tile_pool(name="hsB", bufs=2) as hsb,
                tc.tile_pool(name="psQ", bufs=7, space="PSUM") as psQ,
                tc.tile_pool(name="ropeq", bufs=2) as rqp,
            ):
                wq_sb = wqp.tile([128, 16, HPG * D_Q], f32r)
                for k in range(16):
                    nc.sync.dma_start(wq_sb[:, k], wqT_r[:, k])
                for nt in range(NT):
                    nts = slice(nt * 512, (nt + 1) * 512)
                    pms = [psQ.tile([128, 512], f32, tag="pq", name=f"pq{m}")
                           for m in range(6)]
                    for ko in range(4):
                        hq = hsb.tile([128, 4, 512], f32r, tag="hs2")
                        for kk in range(4):
                            nc.sync.dma_start(hq[:, kk], hsT_r[:, ko * 4 + kk, nts])
                        for kk in range(4):
                            k = ko * 4 + kk
                            for m in range(6):
                                nc.tensor.matmul(pms[m][:], wq_sb[:, k, m * 128:(m + 1) * 128],
                                                 hq[:, kk], start=(k == 0), stop=(k == 15))
                    for m in range(6):
                        nc.scalar.activation(qT[:, m, nts], pms[m][:], FT.Copy, scale=SCALE)
                # RoPE on the two pe chunks (4: heads 0,1; 5: heads 2,3)
                for c in (4, 5):
                    rq = rqp.tile([128, S], f32, tag="rq")
                    nc.scalar.copy(rq[0:32], qT[32:64, c, :])
                    nc.scalar.copy(rq[32:64], qT[0:32, c, :])
                    nc.scalar.copy(rq[64:96], qT[96:128, c, :])
                    nc.scalar.copy(rq[96:128], qT[64:96, c, :])
                    nc.vector.tensor_scalar_mul(qT[:, c, :], qT[:, c, :], c_cs[:, 0:1])
                    nc.vector.tensor_scalar_mul(rq[:], rq[:], c_cs[:, 1:2])
                    nc.vector.tensor_add(qT[:, c, :], qT[:, c, :], rq[:])

            # ---------- Phase B2: causal attention ----------
            with (  # noqa: SIM117
                _maybe(_PHASES >= 3) as _go3,
            ):
             if _go3:
              with (
                tc.tile_pool(name="knp", bufs=1) as knp,
                tc.tile_pool(name="vp", bufs=1) as vp,
                tc.tile_pool(name="pTp", bufs=1) as pTp,
                tc.tile_pool(name="bcsp", bufs=2) as bcsp,
                tc.tile_pool(name="smp", bufs=2) as smp,
                tc.tile_pool(name="aosp", bufs=2) as aosp,
                tc.tile_pool(name="psS", bufs=3, space="PSUM") as psS,
                tc.tile_pool(name="psAV", bufs=2, space="PSUM") as psAV,
                tc.tile_pool(name="psSE", bufs=2, space="PSUM") as psSE,
                tc.tile_pool(name="psBC", bufs=1, space="PSUM") as psBC,
            ):
                knope_sb = knp.tile([128, HPG, S], f32r)
                for m in range(HPG):
                    nc.sync.dma_start(knope_sb[:, m], knope_dram[:, m])
                v_sb = vp.tile([128, TT, HPG * D_V], f32r)
                for tt in range(0, TT, 4):
                    nc.sync.dma_start(v_sb[:, tt:tt + 4], v_dram[:, tt:tt + 4])
                for it in range(NT):
                    its = slice(it * 512, (it + 1) * 512)
                    njt = 4 * it + 4
                    for h in range(HPG):
                        pT = pTp.tile([128, TT, 512], f32r, tag="pT")
                        qpe = qT[64 * (h % 2):64 * (h % 2) + 64, 4 + h // 2, its]
                        se = psSE.tile([1, 512], f32, tag="se")
                        av = psAV.tile([128, 512], f32, tag="av")

                        def acc_jt(jt):
                            nc.tensor.matmul(se[:], c_onec[:], pT[:, jt],
                                             start=(jt == 0), stop=(jt == njt - 1))
                            nc.tensor.matmul(av[:], v_sb[:, jt, h * 128:(h + 1) * 128],
                                             pT[:, jt],
                                             start=(jt == 0), stop=(jt == njt - 1))

                        for jt in range(njt):
                            jts = slice(jt * 128, (jt + 1) * 128)
                            sT = psS.tile([128, 512], f32, tag="sT")
                            nc.tensor.matmul(sT[:], knope_sb[:, h, jts], qT[:, h, its],
                                             start=True, stop=False)
                            pb = 64 * (h % 2)
                            nc.tensor.matmul(sT[:], kpeT[pb:pb + 64, jts], qpe,
                                             start=False, stop=True)
                            nc.scalar.activation(pT[:, jt], sT[:], FT.Exp)
                            kd = jt - 4 * it
                            if kd >= 0:  # diagonal-crossing tile: causal mask
                                nc.vector.tensor_tensor(pT[:, jt], pT[:, jt],
                                                        c_masks[:, kd, :], OP.mult)
                            if jt >= 1:
                                acc_jt(jt - 1)
                        acc_jt(njt - 1)

                        rinv = smp.tile([1, 512], f32r, tag="ri")
                        nc.vector.reciprocal(rinv[:], se[:])
                        bc = psBC.tile([128, 512], f32, tag="bc2")
                        nc.tensor.matmul(bc[:], c_oner[:], rinv[:], start=True, stop=True)
                        bcs = bcsp.tile([128, 512], f32, tag="bcs")
                        nc.scalar.copy(bcs[:], bc[:])
                        ao = aosp.tile([128, 512], f32r, tag="ao")
                        nc.vector.tensor_tensor(ao[:], av[:], bcs[:], OP.mult)
                        nc.sync.dma_start(ao_dram[:, h, its], ao[:])

            # ---------- Phase B3: output projection (partial) ----------
            with (  # noqa: SIM117
                _maybe(_PHASES >= 4) as _go4,
            ):
             if _go4:
              with (
                tc.tile_pool(name="wo", bufs=1) as wop,
                tc.tile_pool(name="outs", bufs=3) as osp,
                tc.tile_pool(name="psO", bufs=2, space="PSUM") as psO,
            ):
                wo_sb = wop.tile([128, 4, HID], f32r, tag="wo")
                ao_sb = wop.tile([128, 4, S], f32r, tag="aor")
                for c in range(4):
                    nc.sync.dma_start(wo_sb[:, c], woT_r[:, c])
                    nc.sync.dma_start(ao_sb[:, c], ao_dram[:, c])
                for tt in range(TT):
                    tts = slice(tt * 128, (tt + 1) * 128)
                    for ot in range(4):
                        ots = slice(ot * 512, (ot + 1) * 512)
                        po = psO.tile([128, 512], f32, tag="po")
                        for c in range(4):
                            nc.tensor.matmul(po[:], ao_sb[:, c, tts], wo_sb[:, c, ots],
                                             start=(c == 0), stop=(c == 3))
                        ob = osp.tile([128, 512], f32, tag="ob")
                        nc.scalar.copy(ob[:], po[:])
                        nc.sync.dma_start(outp[tts, ots], ob[:])

    nc.compile()
    return nc


def _get_compiled():
    global _compiled
    if _compiled is None:
        _compiled = _build()
    return _compiled


def _host_prep(hidden_states, Wq, Wkva, kv_a_norm_weight, Wkvb, Wo, cos, sin):
    hs = np.asarray(hidden_states, dtype=np.float32)
    Wq = np.asarray(Wq, dtype=np.float32)
    Wkva = np.asarray(Wkva, dtype=np.float32)
    w_norm = np.asarray(kv_a_norm_weight, dtype=np.float32)
    Wkvb = np.asarray(Wkvb, dtype=np.float32)
    Wo = np.asarray(Wo, dtype=np.float32)
    cos64 = np.asarray(cos, dtype=np.float32).reshape(D_ROPE)
    sin64 = np.asarray(sin, dtype=np.float32).reshape(D_ROPE)

    wkvaT = np.ascontiguousarray(Wkva.T)                       # [HID, 576]
    normw = np.ascontiguousarray(w_norm.reshape(4, 128).T)     # [128, 4]
    # rotate_half folded into the sin vector: first half gets -sin
    s2 = np.concatenate([-sin64[:32], sin64[32:]])
    cs_host = np.ascontiguousarray(
        np.stack([np.tile(cos64, 2), np.tile(s2, 2)], axis=1))  # [128, 2]
    jj = np.arange(128)[:, None, None]
    kd = np.arange(4)[None, :, None]
    ii = np.arange(512)[None, None, :]
    masks_host = (kd * 128 + jj <= ii).astype(ml_dtypes.bfloat16)  # [128, 4, 512]
    onec = np.ones((128, 1), dtype=np.float32)
    oner = np.ones((1, 128), dtype=np.float32)

    hsTs = [np.ascontiguousarray(hs[b].T) for b in range(B)]

    in_maps = []
    for core in range(N_CORES):
        b, g = divmod(core, G)
        heads = list(range(g * HPG, (g + 1) * HPG))
        wq_rows = np.concatenate(
            [Wq[h * D_Q:h * D_Q + D_NOPE] for h in heads]
            + [Wq[h * D_Q + D_NOPE:(h + 1) * D_Q] for h in heads], axis=0)
        wqT = np.ascontiguousarray(wq_rows.T)                  # [HID, 768]
        wkvbkT = np.ascontiguousarray(np.concatenate(
            [Wkvb[h * 256:h * 256 + 128] for h in heads], axis=0).T)   # [LORA, 512]
        wkvbvT = np.ascontiguousarray(np.concatenate(
            [Wkvb[h * 256 + 128:h * 256 + 256] for h in heads], axis=0).T)
        woT = np.ascontiguousarray(np.concatenate(
            [Wo[:, h * D_V:(h + 1) * D_V] for h in heads], axis=1).T)  # [512, HID]
        in_maps.append({
            "hsT": hsTs[b], "wqT": wqT, "wkvaT": wkvaT,
            "wkvbkT": wkvbkT, "wkvbvT": wkvbvT, "woT": woT,
            "normw": normw, "cs": cs_host, "masks": masks_host,
            "onec": onec, "oner": oner,
        })
    return in_maps


def _install_ntff_hook():
    """Register the axon NTFF profiling hook (missing antenv.axon_hooks stub)."""
    import types

    if "antenv.axon_hooks" in sys.modules:
        return
    import antenv  # noqa: F401
    mod = types.ModuleType("antenv.axon_hooks")
    mod._hook = None
    mod.set_axon_ntff_profile_hook = lambda h: setattr(mod, "_hook", h)
    mod.get_axon_ntff_profile_hook = lambda: mod._hook
    sys.modules["antenv.axon_hooks"] = mod
    try:
        from trn_agent_boot.trn_boot import _ntff_profile_via_ctypes
        mod._hook = _ntff_profile_via_ctypes("/opt/axon/libaxon_pjrt.so")
    except Exception as e:  # profiling is best-effort
        print(f"ntff hook install failed: {e}")


def kernel(hidden_states, Wq, Wkva, kv_a_norm_weight, Wkvb, Wo, cos, sin):
    in_maps = _host_prep(hidden_states, Wq, Wkva, kv_a_norm_weight,
                         Wkvb, Wo, cos, sin)
    if TRACE:
        _install_ntff_hook()
    nc = _get_compiled()
    res = run_bass_kernel_spmd(nc, in_maps, core_ids=list(range(N_CORES)),
                               trace=TRACE)
    kernel.last_result = res
    out = np.zeros((B, S, HID), dtype=np.float32)
    for core in range(N_CORES):
        b = core // G
        out[b] += res.results[core]["outp"]
    return out


# revision 20
# speedup vs baseline: 1.1409x; 1.1409x over previous
"""DeepseekV2-Lite MLA-vanilla attention block on 8 Trainium2 NeuronCores.

Sharding: tensor-parallel over the 16 heads (4 groups of 4 heads) x
data-parallel over batch (2) -> 8 cores. The kv_a (compressed latent) path is
replicated within a batch. Each core computes a partial output
(its 4 heads' contribution through Wo); the host sums the 4 partials per batch.

All on-device layouts are feature-major ("transposed") so every matmul
contracts over the SBUF partition dimension:
  - hsT [HID, S], wqT [HID, 768], ... prepared host-side.
  - scores are computed transposed, sT[j, i] = k . q, so softmax row sums are
    ones-vector matmuls and the causal mask is 4 static diagonal tiles.
  - softmax skips max-subtraction (scores are O(1) for these inputs; exp is
    computed in fp32 which is safe up to ~80).
  - reciprocals happen AFTER broadcasting row sums to [128, 512] so the PE
    never waits on a serial [1,512] reciprocal; per-head finalization is
    emitted one head late so the PE instruction queue never stalls (stalls
    >3.4us re-throttle the PE clock to 1.2GHz).
Matmuls run as float32r (measured same issue rate as bf16 at N=512);
attention probabilities and V run in bf16.

Phase order: q-proj -> ckv-proj+RMSNorm -> kv_b -> attention -> Wo, chosen so
weight prefetches overlap the previous phase's compute.
"""

import os
import sys
from contextlib import contextmanager

sys.path.insert(0, "/opt/trn_rl_repo")

_PHASES = int(os.environ.get("KPHASES", "5"))  # debug: 1=B1, 2=+A1, 3=+A2, 4=+B2, 5=all


@contextmanager
def _maybe(cond):
    yield cond


import numpy as np
import ml_dtypes

import concourse.bass as bass  # noqa: F401
import concourse.mybir as mybir
import concourse.tile as tile
from concourse import bacc
from concourse.bass_utils import run_bass_kernel_spmd

B, S, HID = 2, 2048, 2048
NH, D_NOPE, D_ROPE, D_Q, D_V, LORA = 16, 128, 64, 192, 128, 512
SCALE = D_Q ** -0.5
EPS = 1e-6
G = 4          # head groups (tensor parallel)
HPG = NH // G  # heads per group
N_CORES = 8
NT = S // 512  # 512-token tiles
TT = S // 128  # 128-token tiles

TRACE = False  # set by test.py to capture an NTFF profile

f32 = mybir.dt.float32
f32r = mybir.dt.float32r
bf16 = mybir.dt.bfloat16

_compiled = None


def _build():
    FT = mybir.ActivationFunctionType
    OP = mybir.AluOpType

    nc = bacc.Bacc("TRN2", target_bir_lowering=False, debug=False,
                   num_devices=N_CORES)

    hsT = nc.dram_tensor("hsT", [HID, S], f32r, kind="ExternalInput").ap()
    wqT = nc.dram_tensor("wqT", [HID, HPG * D_Q], f32r, kind="ExternalInput").ap()
    wkvaT = nc.dram_tensor("wkvaT", [HID, LORA + D_ROPE], f32r, kind="ExternalInput").ap()
    wkvbkT = nc.dram_tensor("wkvbkT", [LORA, HPG * D_NOPE], f32r, kind="ExternalInput").ap()
    wkvbvT = nc.dram_tensor("wkvbvT", [LORA, HPG * D_V], f32r, kind="ExternalInput").ap()
    woT = nc.dram_tensor("woT", [HPG * D_V, HID], f32r, kind="ExternalInput").ap()
    cs = nc.dram_tensor("cs", [128, 2], f32, kind="ExternalInput").ap()
    masks = nc.dram_tensor("masks", [128, 4, 512], bf16, kind="ExternalInput").ap()
    onec = nc.dram_tensor("onec", [128, 1], f32r, kind="ExternalInput").ap()
    onecb = nc.dram_tensor("onecb", [128, 1], bf16, kind="ExternalInput").ap()
    oner = nc.dram_tensor("oner", [1, 128], f32r, kind="ExternalInput").ap()
    outp = nc.dram_tensor("outp", [S, HID], f32, kind="ExternalOutput").ap()

    hsT_r = hsT.rearrange("(ko p) t -> p ko t", p=128)        # [128, 16, S]
    wqT_r = wqT.rearrange("(ko p) f -> p ko f", p=128)        # [128, 16, 768]
    wkvaT_r = wkvaT.rearrange("(ko p) f -> p ko f", p=128)    # [128, 16, 576]
    wkvbkT_r = wkvbkT.rearrange("(c p) f -> p c f", p=128)    # [128, 4, 512]
    wkvbvT_r = wkvbvT.rearrange("(c p) f -> p c f", p=128)    # [128, 4, 512]
    woT_r = woT.rearrange("(c p) o -> p c o", p=128)          # [128, 4, HID]

    with tile.TileContext(nc) as tc, nc.allow_low_precision(
        reason="float32r/bf16 rounding of matmul operands is the design"
    ):
        with (
            tc.tile_pool(name="dram", bufs=1, space="DRAM") as dram,
            tc.tile_pool(name="const", bufs=1) as const,
            tc.tile_pool(name="keep", bufs=1) as keep,
        ):
            knope_dram = dram.tile([128, HPG, S], f32r)   # [d_nope, head, t]
            v_dram = dram.tile([128, TT, HPG * D_V], bf16)  # [tok, tt, dv]
            ao_dram = dram.tile([128, HPG, S], f32r)      # [d_v, head, t]

            c_onec = const.tile([128, 1], f32r)
            nc.sync.dma_start(c_onec[:], onec)
            c_onecb = const.tile([128, 1], bf16)
            nc.sync.dma_start(c_onecb[:], onecb)
            c_oner = const.tile([1, 128], f32r)
            nc.sync.dma_start(c_oner[:], oner)
            c_cs = const.tile([128, 2], f32)
            nc.sync.dma_start(c_cs[:], cs)
            c_masks = const.tile([128, 4, 512], bf16)
            nc.sync.dma_start(c_masks[:], masks)
            c_eps = const.tile([1, 1], f32)
            nc.gpsimd.memset(c_eps[:], EPS)

            # k_pe stored twice (partitions 0:64 and 64:128) so the scores
            # matmul lhsT base_partition can match either q_pe half.
            kpeT = keep.tile([128, S], f32r, tag="kpeT")
            qT = keep.tile([128, 6, S], f32r, tag="qT")

            # hs tiles shared by the q and ckv projections (pool spans both
            # phases so the ckv phase's prefetch overlaps q-proj compute).
            with tc.tile_pool(name="hsp", bufs=2) as hsp:

                def load_hq(nt, ko):
                    nts = slice(nt * 512, (nt + 1) * 512)
                    hq = hsp.tile([128, 4, 512], f32r, tag="hs", name="hq")
                    for kk in range(4):
                        nc.sync.dma_start(hq[:, kk], hsT_r[:, ko * 4 + kk, nts])
                    return hq

                # ---------- Phase B1: q projection (+ scale + RoPE) ----------
                with (
                    tc.tile_pool(name="wq", bufs=1) as wqp,
                    tc.tile_pool(name="psQ", bufs=7, space="PSUM") as psQ,
                    tc.tile_pool(name="ropeq", bufs=2) as rqp,
                ):
                    wq_sb = wqp.tile([128, 16, HPG * D_Q], f32r)
                    for k in range(16):
                        nc.sync.dma_start(wq_sb[:, k], wqT_r[:, k])
                    for nt in range(NT):
                        nts = slice(nt * 512, (nt + 1) * 512)
                        pms = [psQ.tile([128, 512], f32, tag="pq", name=f"pq{m}")
                               for m in range(6)]
                        for ko in range(4):
                            hq = load_hq(nt, ko)
                            for kk in range(4):
                                k = ko * 4 + kk
                                for m in range(6):
                                    nc.tensor.matmul(
                                        pms[m][:], wq_sb[:, k, m * 128:(m + 1) * 128],
                                        hq[:, kk], start=(k == 0), stop=(k == 15))
                        for m in range(6):
                            nc.scalar.activation(qT[:, m, nts], pms[m][:],
                                                 FT.Copy, scale=SCALE)
                        # RoPE on the pe chunks (4: heads 0,1; 5: heads 2,3),
                        # per n-tile so it trails under later matmuls.
                        for c in (4, 5):
                            rq = rqp.tile([128, 512], f32, tag="rq", name="rq")
                            nc.scalar.copy(rq[0:32], qT[32:64, c, nts])
                            nc.scalar.copy(rq[32:64], qT[0:32, c, nts])
                            nc.scalar.copy(rq[64:96], qT[96:128, c, nts])
                            nc.scalar.copy(rq[96:128], qT[64:96, c, nts])
                            nc.vector.tensor_scalar_mul(qT[:, c, nts], qT[:, c, nts],
                                                        c_cs[:, 0:1])
                            nc.vector.tensor_scalar_mul(rq[:], rq[:], c_cs[:, 1:2])
                            nc.vector.tensor_add(qT[:, c, nts], qT[:, c, nts], rq[:])

                # ---------- Phase A1: ckv projection + in-place RMSNorm ----
                M_CKV = ((0, 128), (128, 128), (256, 128), (384, 128), (512, 64))
                with (  # noqa: SIM117
                    _maybe(_PHASES >= 2) as _go2,
                ):
                 if _go2:
                  with tc.tile_pool(name="ckv", bufs=1) as ckvp:
                    ckv = ckvp.tile([128, 5, S], f32r)
                    with (
                        tc.tile_pool(name="wkva", bufs=1) as wk,
                        tc.tile_pool(name="ntmp", bufs=2) as ntp,
                        tc.tile_pool(name="psA", bufs=5, space="PSUM") as psA,
                        tc.tile_pool(name="psN", bufs=1, space="PSUM") as psN,
                        tc.tile_pool(name="psNb", bufs=1, space="PSUM") as psNb,
                    ):
                        wkva_sb = wk.tile([128, 16, 576], f32r)
                        for k in range(16):
                            nc.sync.dma_start(wkva_sb[:, k], wkvaT_r[:, k])

                        def norm_nt(nt):
                            # RMS-normalize ckv chunks 0..3 for this n-tile,
                            # in place. Broadcast-then-reciprocal keeps the
                            # serial reciprocal off the PE's critical path.
                            nts = slice(nt * 512, (nt + 1) * 512)
                            ssq = psN.tile([1, 512], f32, tag="ssq", name="ssq")
                            for c in range(4):
                                sq = ntp.tile([128, 512], f32r, tag="sq", name="sq")
                                nc.scalar.activation(sq[:], ckv[:, c, nts], FT.Square)
                                nc.tensor.matmul(ssq[:], c_onec[:], sq[:],
                                                 start=(c == 0), stop=(c == 3))
                            rms = ntp.tile([1, 512], f32r, tag="rms", name="rms")
                            nc.scalar.activation(rms[:], ssq[:], FT.Sqrt,
                                                 scale=1.0 / LORA, bias=c_eps[:])
                            bc = psNb.tile([128, 512], f32, tag="bc", name="bc")
                            nc.tensor.matmul(bc[:], c_oner[:], rms[:],
                                             start=True, stop=True)
                            rbc = ntp.tile([128, 512], f32, tag="rbc", name="rbc")
                            nc.vector.reciprocal(rbc[:], bc[:])
                            for c in range(4):
                                nc.vector.tensor_tensor(ckv[:, c, nts],
                                                        ckv[:, c, nts], rbc[:],
                                                        OP.mult)

                        pending_norm = None
                        for nt in range(NT):
                            nts = slice(nt * 512, (nt + 1) * 512)
                            pms = [psA.tile([128, 512], f32, tag="pa", name=f"pa{m}")
                                   for m in range(5)]
                            for ko in range(4):
                                hq = load_hq(nt, ko)
                                if ko == 1 and pending_norm is not None:
                                    pending_norm()
                                    pending_norm = None
                                for kk in range(4):
                                    k = ko * 4 + kk
                                    for m, (mo, mw) in enumerate(M_CKV):
                                        nc.tensor.matmul(
                                            pms[m][:mw], wkva_sb[:, k, mo:mo + mw],
                                            hq[:, kk], start=(k == 0), stop=(k == 15))
                            for m, (mo, mw) in enumerate(M_CKV):
                                nc.scalar.copy(ckv[:mw, m, nts], pms[m][:mw])
                            pending_norm = (lambda nt=nt: norm_nt(nt))
                        pending_norm()

                        # RoPE on k_pe (rows 0:64 of chunk 4)
                        rk = ntp.tile([64, S], f32, tag="ropek", name="rk")
                        nc.scalar.copy(rk[0:32], ckv[32:64, 4, :])
                        nc.scalar.copy(rk[32:64], ckv[0:32, 4, :])
                        nc.vector.tensor_scalar_mul(kpeT[0:64], ckv[0:64, 4, :],
                                                    c_cs[0:64, 0:1])
                        nc.vector.tensor_scalar_mul(rk[:], rk[:], c_cs[0:64, 1:2])
                        nc.vector.tensor_add(kpeT[0:64], kpeT[0:64], rk[:])
                        nc.scalar.copy(kpeT[64:128], kpeT[0:64])

                    # ---------- Phase A2: kv_b projections ----------
                    with (  # noqa: SIM117
                        _maybe(_PHASES >= 3) as _go3,
                    ):
                     if _go3:
                      with (
                        tc.tile_pool(name="wkvb", bufs=1) as wbp,
                        tc.tile_pool(name="kvst", bufs=3) as kst,
                        tc.tile_pool(name="psB", bufs=2, space="PSUM") as psB,
                    ):
                        wbk = wbp.tile([128, 4, 512], f32r, tag="wbk")
                        wbv = wbp.tile([128, 4, 512], f32r, tag="wbv")
                        for c in range(4):
                            nc.sync.dma_start(wbk[:, c], wkvbkT_r[:, c])
                            nc.sync.dma_start(wbv[:, c], wkvbvT_r[:, c])
                        # k_nope, feature-major: [d_nope, head, t]
                        for nt in range(NT):
                            nts = slice(nt * 512, (nt + 1) * 512)
                            for m in range(HPG):
                                pm = psB.tile([128, 512], f32, tag="pb", name="pm")
                                for c in range(4):
                                    nc.tensor.matmul(
                                        pm[:], wbk[:, c, m * 128:(m + 1) * 128],
                                        ckv[:, c, nts], start=(c == 0), stop=(c == 3))
                                ks = kst.tile([128, 512], f32r, tag="kn", name="ks")
                                nc.scalar.copy(ks[:], pm[:])
                                nc.sync.dma_start(knope_dram[:, m, nts], ks[:])
                        # v, token-major: [tok, tt, dv], bf16
                        for tt in range(TT):
                            tts = slice(tt * 128, (tt + 1) * 128)
                            pv = psB.tile([128, 512], f32, tag="pb", name="pv")
                            for c in range(4):
                                nc.tensor.matmul(pv[:], ckv[:, c, tts], wbv[:, c, :],
                                                 start=(c == 0), stop=(c == 3))
                            vs = kst.tile([128, 512], bf16, tag="vs", name="vs")
                            nc.scalar.copy(vs[:], pv[:])
                            nc.sync.dma_start(v_dram[:, tt, :], vs[:])

            # ---------- Phase B2: causal attention ----------
            with (  # noqa: SIM117
                _maybe(_PHASES >= 4) as _go4,
            ):
             if _go4:
              with (
                tc.tile_pool(name="wo", bufs=1) as wop,
                tc.tile_pool(name="knp", bufs=1) as knp,
                tc.tile_pool(name="vp", bufs=1) as vp,
                tc.tile_pool(name="pTp", bufs=1) as pTp,
                tc.tile_pool(name="bcsp", bufs=2) as bcsp,
                tc.tile_pool(name="smp", bufs=2) as smp,
                tc.tile_pool(name="aosp", bufs=2) as aosp,
              ):
                # Wo weights prefetch here, overlapping attention compute.
                wo_sb = wop.tile([128, 4, HID], f32r, tag="wo")
                for c in range(4):
                    nc.sync.dma_start(wo_sb[:, c], woT_r[:, c])

                with (
                    tc.tile_pool(name="psS", bufs=3, space="PSUM") as psS,
                    tc.tile_pool(name="psAV", bufs=2, space="PSUM") as psAV,
                    tc.tile_pool(name="psSE", bufs=2, space="PSUM") as psSE,
                    tc.tile_pool(name="psBC", bufs=1, space="PSUM") as psBC,
                ):
                    knope_sb = knp.tile([128, HPG, S], f32r)
                    for m in range(HPG):
                        nc.sync.dma_start(knope_sb[:, m], knope_dram[:, m])
                    v_sb = vp.tile([128, TT, HPG * D_V], bf16)
                    for tt in range(0, TT, 2):
                        nc.sync.dma_start(v_sb[:, tt:tt + 2], v_dram[:, tt:tt + 2])

                    def make_fin(se, av, h, its):
                        def fin():
                            se_sb = smp.tile([1, 512], f32r, tag="ses", name="se_sb")
                            nc.scalar.copy(se_sb[:], se[:])
                            bc = psBC.tile([128, 512], f32, tag="bc2", name="bc2")
                            nc.tensor.matmul(bc[:], c_oner[:], se_sb[:],
                                             start=True, stop=True)
                            rbc = bcsp.tile([128, 512], f32, tag="rbc2", name="rbc2")
                            nc.vector.reciprocal(rbc[:], bc[:])
                            ao = aosp.tile([128, 512], f32r, tag="ao", name="ao")
                            nc.vector.tensor_tensor(ao[:], av[:], rbc[:], OP.mult)
                            nc.sync.dma_start(ao_dram[:, h, its], ao[:])
                        return fin

                    pending_fin = None
                    for it in range(NT):
                        its = slice(it * 512, (it + 1) * 512)
                        njt = 4 * it + 4
                        for h in range(HPG):
                            pT = pTp.tile([128, TT, 512], bf16, tag="pT", name="pT")
                            qpe = qT[64 * (h % 2):64 * (h % 2) + 64, 4 + h // 2, its]
                            se = psSE.tile([1, 512], f32, tag="se", name="se")
                            av = psAV.tile([128, 512], f32, tag="av", name="av")

                            def acc_jt(jt, se=se, av=av, pT=pT, h=h, njt=njt):
                                nc.tensor.matmul(se[:], c_onecb[:], pT[:, jt],
                                                 start=(jt == 0), stop=(jt == njt - 1))
                                nc.tensor.matmul(av[:],
                                                 v_sb[:, jt, h * 128:(h + 1) * 128],
                                                 pT[:, jt],
                                                 start=(jt == 0), stop=(jt == njt - 1))

                            for jt in range(njt):
                                jts = slice(jt * 128, (jt + 1) * 128)
                                sT = psS.tile([128, 512], f32, tag="sT", name="sT")
                                nc.tensor.matmul(sT[:], knope_sb[:, h, jts],
                                                 qT[:, h, its], start=True, stop=False)
                                pb = 64 * (h % 2)
                                nc.tensor.matmul(sT[:], kpeT[pb:pb + 64, jts], qpe,
                                                 start=False, stop=True)
                                nc.scalar.activation(pT[:, jt], sT[:], FT.Exp)
                                kd = jt - 4 * it
                                if kd >= 0:  # diagonal tile: causal mask
                                    nc.vector.tensor_tensor(pT[:, jt], pT[:, jt],
                                                            c_masks[:, kd, :], OP.mult)
                                if jt == 1 and pending_fin is not None:
                                    pending_fin()
                                    pending_fin = None
                                if jt >= 1:
                                    acc_jt(jt - 1)
                            acc_jt(njt - 1)
                            pending_fin = make_fin(se, av, h, its)
                    pending_fin()

                # ---------- Phase B3: output projection (partial) ----------
                with (  # noqa: SIM117
                    _maybe(_PHASES >= 5) as _go5,
                ):
                 if _go5:
                  with (
                    tc.tile_pool(name="aop", bufs=1) as aop,
                    tc.tile_pool(name="outs", bufs=3) as osp,
                    tc.tile_pool(name="psO", bufs=2, space="PSUM") as psO,
                ):
                    ao_sb = aop.tile([128, 4, S], f32r)
                    for c in range(4):
                        nc.sync.dma_start(ao_sb[:, c], ao_dram[:, c])
                    for tt in range(TT):
                        tts = slice(tt * 128, (tt + 1) * 128)
                        for ot in range(4):
                            ots = slice(ot * 512, (ot + 1) * 512)
                            po = psO.tile([128, 512], f32, tag="po", name="po")
                            for c in range(4):
                                nc.tensor.matmul(po[:], ao_sb[:, c, tts],
                                                 wo_sb[:, c, ots],
                                                 start=(c == 0), stop=(c == 3))
                            ob = osp.tile([128, 512], f32, tag="ob", name="ob")
                            nc.scalar.copy(ob[:], po[:])
                            nc.sync.dma_start(outp[tts, ots], ob[:])

    nc.compile()
    return nc


def _get_compiled():
    global _compiled
    if _compiled is None:
        _compiled = _build()
    return _compiled


def _host_prep(hidden_states, Wq, Wkva, kv_a_norm_weight, Wkvb, Wo, cos, sin):
    hs = np.asarray(hidden_states, dtype=np.float32)
    Wq = np.asarray(Wq, dtype=np.float32)
    Wkva = np.asarray(Wkva, dtype=np.float32)
    w_norm = np.asarray(kv_a_norm_weight, dtype=np.float32)
    # fold the RMSNorm weight into the kv_b weight columns (per latent channel)
    Wkvb = np.asarray(Wkvb, dtype=np.float32) * w_norm[None, :]
    Wo = np.asarray(Wo, dtype=np.float32)
    cos64 = np.asarray(cos, dtype=np.float32).reshape(D_ROPE)
    sin64 = np.asarray(sin, dtype=np.float32).reshape(D_ROPE)

    wkvaT = np.ascontiguousarray(Wkva.T)                       # [HID, 576]
    # rotate_half folded into the sin vector: first half gets -sin
    s2 = np.concatenate([-sin64[:32], sin64[32:]])
    cs_host = np.ascontiguousarray(
        np.stack([np.tile(cos64, 2), np.tile(s2, 2)], axis=1))  # [128, 2]
    jj = np.arange(128)[:, None, None]
    kd = np.arange(4)[None, :, None]
    ii = np.arange(512)[None, None, :]
    masks_host = (kd * 128 + jj <= ii).astype(ml_dtypes.bfloat16)  # [128, 4, 512]
    onec = np.ones((128, 1), dtype=np.float32)
    onecb = np.ones((128, 1), dtype=ml_dtypes.bfloat16)
    oner = np.ones((1, 128), dtype=np.float32)

    hsTs = [np.ascontiguousarray(hs[b].T) for b in range(B)]

    in_maps = []
    for core in range(N_CORES):
        b, g = divmod(core, G)
        heads = list(range(g * HPG, (g + 1) * HPG))
        wq_rows = np.concatenate(
            [Wq[h * D_Q:h * D_Q + D_NOPE] for h in heads]
            + [Wq[h * D_Q + D_NOPE:(h + 1) * D_Q] for h in heads], axis=0)
        wqT = np.ascontiguousarray(wq_rows.T)                  # [HID, 768]
        wkvbkT = np.ascontiguousarray(np.concatenate(
            [Wkvb[h * 256:h * 256 + 128] for h in heads], axis=0).T)   # [LORA, 512]
        wkvbvT = np.ascontiguousarray(np.concatenate(
            [Wkvb[h * 256 + 128:h * 256 + 256] for h in heads], axis=0).T)
        woT = np.ascontiguousarray(np.concatenate(
            [Wo[:, h * D_V:(h + 1) * D_V] for h in heads], axis=1).T)  # [512, HID]
        in_maps.append({
            "hsT": hsTs[b], "wqT": wqT, "wkvaT": wkvaT,
            "wkvbkT": wkvbkT, "wkvbvT": wkvbvT, "woT": woT,
            "cs": cs_host, "masks": masks_host,
            "onec": onec, "onecb": onecb, "oner": oner,
        })
    return in_maps


def _install_ntff_hook():
    """Register the axon NTFF profiling hook (missing antenv.axon_hooks stub)."""
    import types

    if "antenv.axon_hooks" in sys.modules:
        return
    import antenv  # noqa: F401
    mod = types.ModuleType("antenv.axon_hooks")
    mod._hook = None
    mod.set_axon_ntff_profile_hook = lambda h: setattr(mod, "_hook", h)
    mod.get_axon_ntff_profile_hook = lambda: mod._hook
    sys.modules["antenv.axon_hooks"] = mod
    try:
        from trn_agent_boot.trn_boot import _ntff_profile_via_ctypes
        mod._hook = _ntff_profile_via_ctypes("/opt/axon/libaxon_pjrt.so")
    except Exception as e:  # profiling is best-effort
        print(f"ntff hook install failed: {e}")


def kernel(hidden_states, Wq, Wkva, kv_a_norm_weight, Wkvb, Wo, cos, sin):
    in_maps = _host_prep(hidden_states, Wq, Wkva, kv_a_norm_weight,
                         Wkvb, Wo, cos, sin)
    if TRACE:
        _install_ntff_hook()
    nc = _get_compiled()
    res = run_bass_kernel_spmd(nc, in_maps, core_ids=list(range(N_CORES)),
                               trace=TRACE)
    kernel.last_result = res
    out = np.zeros((B, S, HID), dtype=np.float32)
    for core in range(N_CORES):
        b = core // G
        out[b] += res.results[core]["outp"]
    return out
